# revision 17
# baseline (speedup 1.0000x reference)
"""AFT-Full (nn_AFT_Full) Trainium2 Bass kernel, 8-core SPMD, batch-sharded.

Math note: in the reference, w_bias has shape [1,T,T] and max over dim 0 is the
identity, so exp_wb == exp(0) == 1 and the [T,T] matmuls reduce to column sums
over T (u/vp are unused):
    num[b,h] = sum_t exp(k[b,t,h] - m[t,h]) * v[b,t,h]
    den[b,h] = sum_t exp(k[b,t,h] - m[t,h])
    out = (sigmoid(q) * num/den) @ Wo + bo
where m[t,h] = max over the FULL batch of k -> cross-core AllReduce.

We compute E0 = exp(k + bk) directly, take M = max_b E0 (exp is monotone),
s = 1/M per core, AllReduce(min) on s, and use e = E0 * s.
bv is folded out of the v projection: num/den + bv == r with raw v.

Design (all-bf16: num = sum_t e*v is a cancelling sum, so ANY per-element
noise -- fp8 storage or fp8 matmul inputs -- passes into r at full strength;
only bf16 keeps the 2e-2 budget):
 - x transposed on PE once; xt kept bf16-resident for all 9 windows (no DRAM
   round trip for pass-2 q projections).
 - Projections run weights-stationary over window groups of 3 (one LDWEIGHTS
   feeds 3 matmuls), k and v in pass 1, q in pass 2.
 - V is staged to DRAM in pass 1 and streamed back per (hc,b) during nd --
   that frees 46KB/partition of SBUF so xt and E0 stay resident.
 - num/den via fused tensor_tensor_reduce (elementwise out clobbers E0 in
   place); bv is folded out of the v projection: r = num/den + bv.
 - Pass 2 traces six q projections before any out matmul so the AllReduce
   hides under PE work; nd per batch-chunk gates out windows in order.
"""
import os
import sys

sys.path.insert(0, "/opt/trn_rl_repo")

import numpy as np

# ---- problem constants (hardcoded per spec) ----
B, Hh, Ww, C = 64, 24, 24, 768
HID = 576
T = Hh * Ww          # 576
N_CORES = 8
B_LOC = B // N_CORES  # 8
R = B_LOC * T         # 4608 rows per core
WIN = 512             # row window
NWIN = R // WIN       # 9
NRC = WIN // 128      # 4 row chunks per window
NCC = C // 128        # 6 contraction chunks for projections
HC_SIZES = [128, 128, 128, 128, 64]   # HID = 576 partition chunks
NOUT_HALF = 384       # out matmul free-dim split (768 = 2*384)
WGROUPS = [[0, 1, 2], [3, 4, 5], [6, 7, 8]]
BCHUNKS = [[0, 1, 2], [3, 4, 5], [6, 7]]

_CACHE = {}
LAST_EXEC_NS = None


def _window_segments(w):
    """Batch segments [(b, lo, hi)] of window w, window-local coords."""
    lo, hi = w * WIN, (w + 1) * WIN
    segs = []
    for b in range(B_LOC):
        s = max(lo, b * T)
        e = min(hi, (b + 1) * T)
        if s < e:
            segs.append((b, s - lo, e - lo))
    return segs


def _build():
    import concourse.bass as bass
    import concourse.mybir as mybir
    from concourse import bacc, tile

    f32 = mybir.dt.float32
    bf16 = mybir.dt.bfloat16
    AF = mybir.ActivationFunctionType

    nc = bacc.Bacc("TRN2", target_bir_lowering=False, debug=False,
                   num_devices=N_CORES)

    x = nc.dram_tensor("x", [R, C], f32, kind="ExternalInput").ap()
    Wqb = nc.dram_tensor("Wqb", [128, NCC * HID], bf16, kind="ExternalInput").ap()
    Wkb = nc.dram_tensor("Wkb", [128, NCC * HID], bf16, kind="ExternalInput").ap()
    Wvb = nc.dram_tensor("Wvb", [128, NCC * HID], bf16, kind="ExternalInput").ap()
    bq = nc.dram_tensor("bq", [HID], f32, kind="ExternalInput").ap()
    bk = nc.dram_tensor("bk", [HID], f32, kind="ExternalInput").ap()
    bv = nc.dram_tensor("bv", [HID], f32, kind="ExternalInput").ap()
    Wo = nc.dram_tensor("Wo", [HID, C], f32, kind="ExternalInput").ap()
    bo = nc.dram_tensor("bo", [C], f32, kind="ExternalInput").ap()
    ident = nc.dram_tensor("ident", [128, 128], bf16, kind="ExternalInput").ap()
    out = nc.dram_tensor("out", [R, C], f32, kind="ExternalOutput").ap()

    with tile.TileContext(nc) as tc:
        with (
            tc.tile_pool(name="const", bufs=1) as cpool,
            tc.tile_pool(name="resident", bufs=1) as rpool,
            tc.tile_pool(name="xn", bufs=2) as xnpool,
            tc.tile_pool(name="ob", bufs=2) as obpool,
            tc.tile_pool(name="vst", bufs=3) as vstpool,
            tc.tile_pool(name="vrd", bufs=3) as vrdpool,
            tc.tile_pool(name="scr", bufs=1) as scrpool,
            tc.tile_pool(name="qy", bufs=6) as qypool,
            tc.tile_pool(name="pm", bufs=3, space="PSUM") as pmpool,
            tc.tile_pool(name="pt", bufs=2, space="PSUM") as ptpool,
            tc.tile_pool(name="po", bufs=3, space="PSUM") as popool,
            tc.tile_pool(name="dram", bufs=1, space="DRAM") as dpool,
        ):
            # ---------- constants ----------
            ident_sb = cpool.tile([128, 128], bf16, tag="ident", name="ident")
            nc.sync.dma_start(ident_sb[:], ident[:])

            def load_wb(name, w_ap):
                t = cpool.tile([128, NCC * HID], bf16, tag=name, name=name)
                nc.sync.dma_start(t[:], w_ap[:])
                return t

            def load_wo():
                # Wo extended with bo as an extra contraction row (ones trick)
                tiles = []
                for kc, ksz in enumerate(HC_SIZES):
                    psz = ksz + 1 if kc == 4 else ksz
                    t = cpool.tile([psz, C], bf16, tag=f"Wo_{kc}",
                                   name=f"Wo_{kc}")
                    nc.gpsimd.dma_start(t[0:ksz, :],
                                        Wo[kc * 128:kc * 128 + ksz, :])
                    if kc == 4:
                        nc.gpsimd.dma_start(t[ksz:ksz + 1, :], bo[None, :])
                    tiles.append(t)
                return tiles

            def load_bias(name, b_ap):
                tiles = []
                for hc, hsz in enumerate(HC_SIZES):
                    t = cpool.tile([hsz, 1], f32, tag=f"{name}_{hc}",
                                   name=f"{name}_{hc}")
                    nc.sync.dma_start(t[:], b_ap[hc * 128:hc * 128 + hsz][:, None])
                    tiles.append(t)
                return tiles

            # ---------- resident tensors ----------
            xt_res = [rpool.tile([128, NCC * WIN], bf16, tag=f"xt_{w}",
                                 name=f"xt_{w}") for w in range(NWIN)]
            E0 = [rpool.tile([hsz, R], bf16, tag=f"E0_{hc}", name=f"E0_{hc}")
                  for hc, hsz in enumerate(HC_SIZES)]
            Mx = [rpool.tile([hsz, T], bf16, tag=f"M_{hc}", name=f"M_{hc}")
                  for hc, hsz in enumerate(HC_SIZES)]
            den = [rpool.tile([hsz, B_LOC], f32, tag=f"den_{hc}",
                              name=f"den_{hc}")
                   for hc, hsz in enumerate(HC_SIZES)]
            num = [rpool.tile([hsz, B_LOC], f32, tag=f"num_{hc}",
                              name=f"num_{hc}")
                   for hc, hsz in enumerate(HC_SIZES)]
            rr = [rpool.tile([hsz, B_LOC], f32, tag=f"r_{hc}", name=f"r_{hc}")
                  for hc, hsz in enumerate(HC_SIZES)]

            Wk_sb = load_wb("Wkb", Wkb)
            Wv_sb = load_wb("Wvb", Wvb)
            Wq_sb = load_wb("Wqb", Wqb)
            bk_sb = load_bias("bk", bk)
            bq_sb = load_bias("bq", bq)
            bv_sb = load_bias("bv", bv)
            Wo_sb = load_wo()

            def w3(tile_):
                return tile_[:].rearrange("p (c m) -> p c m", m=HID)

            def xt3(w):
                return xt_res[w][:].rearrange("p (c n) -> p c n", n=WIN)

            # ---------- x load + transpose ----------
            def load_xn(w, queues):
                # f32->bf16 casting DMA must go on gpsimd; split into halves
                # so the two descriptors can ride different DMA channels.
                xn = xnpool.tile([128, NRC * C], bf16, tag="xn", name="xn")
                half = WIN // 2
                for i in range(2):
                    src = x[w * WIN + i * half:w * WIN + (i + 1) * half, :]
                    nc.gpsimd.dma_start(
                        xn[:, i * (half // 128) * C:
                           (i + 1) * (half // 128) * C].rearrange(
                               "p (n c) -> p n c", c=C),
                        src.rearrange("(n p) c -> p n c", p=128))
                return xn

            def transpose_cc(xn, w, cc):
                pt = ptpool.tile([128, WIN], bf16, tag="pt", name="pt")
                for rc in range(NRC):
                    nc.tensor.transpose(
                        pt[:, rc * 128:(rc + 1) * 128],
                        xn[:, rc * C + cc * 128: rc * C + (cc + 1) * 128],
                        ident_sb[:])
                nc.scalar.copy(
                    xt_res[w][:, cc * WIN:(cc + 1) * WIN], pt[:])

            # ---------- DoubleRow projection rounds ----------
            def proj_round(wsb, hc, hsz, wins, pmtag):
                """Project one hid chunk for a group of windows; returns psums.
                Weights stationary: one LDWEIGHTS feeds len(wins) matmuls."""
                pms = [pmpool.tile([hsz, WIN], f32, tag=pmtag, name=pmtag)
                       for _ in wins]
                for cc in range(NCC):
                    lhsT = w3(wsb)[:, cc, hc * 128:hc * 128 + hsz]
                    for i, w in enumerate(wins):
                        nc.tensor.matmul(
                            pms[i][:], lhsT, xt3(w)[:, cc, :],
                            start=(cc == 0), stop=(cc == NCC - 1))
                return pms

            def mx_acc(hc, hsz, w):
                for b, lo, hi in _window_segments(w):
                    t0 = w * WIN + lo - b * T
                    t1 = t0 + (hi - lo)
                    e_seg = E0[hc][:, w * WIN + lo: w * WIN + hi]
                    if b == 0:
                        nc.scalar.copy(Mx[hc][:, t0:t1], e_seg)
                    else:
                        nc.vector.tensor_max(
                            Mx[hc][:, t0:t1], Mx[hc][:, t0:t1], e_seg)

            # V staged in DRAM (frees 46KB/partition of SBUF)
            v_dram = dpool.tile([HID, R], bf16, name="v_dram")

            # bounce buffers for the collective
            bounce_in = dpool.tile([HID, T], bf16, name="bounce_in")
            bounce_out = [dpool.tile([hsz, T], bf16, name=f"bounce_out{hc}",
                                     addr_space="Shared")
                          for hc, hsz in enumerate(HC_SIZES)]

            # ---------- pass 1 ----------
            # xn pool holds 2 windows; loads are paced so a window's buffer
            # is only recycled after its transposes retire.
            xns = {}
            xq = [nc.sync, nc.scalar]
            pending = list(range(5, NWIN))
            xns[0] = load_xn(0, xq)
            xns[1] = load_xn(1, xq)
            for w in (0, 1, 2):
                for cc in range(NCC):
                    transpose_cc(xns[w], w, cc)
                if w + 2 < NWIN:
                    xns[w + 2] = load_xn(w + 2, xq)

            # ---------- pass 1, K phase ----------
            # Mx depends only on the k projections, so all k rounds run
            # first: the per-hc AllReduce then launches ~halfway through
            # pass 1 and hides entirely under the V phase.
            for wgi, wg in enumerate(WGROUPS):
                tq = [(w, cc) for w in WGROUPS[wgi + 1]
                      for cc in range(NCC)] if wgi + 1 < len(WGROUPS) else []
                tqi = 0
                tcounts = [4, 4, 4, 3, 3]
                for hc, hsz in enumerate(HC_SIZES):
                    pks = proj_round(Wk_sb, hc, hsz, wg, "pm")
                    for i, w in enumerate(wg):
                        nc.scalar.activation(
                            E0[hc][:, w * WIN:(w + 1) * WIN], pks[i][:],
                            AF.Exp, bias=bk_sb[hc][:])
                    for _ in range(tcounts[hc]):
                        if tqi < len(tq):
                            w_t, cc_t = tq[tqi]
                            transpose_cc(xns[w_t], w_t, cc_t)
                            tqi += 1
                            if cc_t == NCC - 1 and pending:
                                w_n = pending.pop(0)
                                xns[w_n] = load_xn(w_n, xq)
                    for w in wg:
                        mx_acc(hc, hsz, w)
                    if wgi == len(WGROUPS) - 1:
                        # Mx[hc] final: s = 1/M, bounce out, launch its
                        # AllReduce right away.
                        with nc.allow_low_precision("bf16 softmax scale"):
                            nc.vector.reciprocal(Mx[hc][:], Mx[hc][:])
                        nc.sync.dma_start(
                            bounce_in[hc * 128:hc * 128 + hsz, :], Mx[hc][:])
                        nc.gpsimd.collective_compute(
                            "AllReduce",
                            mybir.AluOpType.min,
                            replica_groups=[list(range(N_CORES))],
                            ins=[bounce_in[hc * 128:hc * 128 + hsz, :].opt()],
                            outs=[bounce_out[hc][:].opt()],
                        )

            Sx = Mx
            for hc, hsz in enumerate(HC_SIZES):
                nc.gpsimd.dma_start(Sx[hc][:], bounce_out[hc][:])

            # ---------- pass 1, V phase (AllReduce + nd hide under it) ----
            def v_wgroup(wg):
                for hc, hsz in enumerate(HC_SIZES):
                    pvs = proj_round(Wv_sb, hc, hsz, wg, "pm")
                    for i, w in enumerate(wg):
                        vt = vstpool.tile([hsz, WIN], bf16, tag="vst",
                                          name="vst")
                        nc.scalar.copy(vt[:], pvs[i][:])
                        nc.sync.dma_start(
                            v_dram[hc * 128:hc * 128 + hsz,
                                   w * WIN:(w + 1) * WIN], vt[:])

            # ---------- pass 2 ----------
            # q projections for six windows run first (the per-hc AllReduce
            # latency hides under them); out windows interleave with the
            # remaining projections so the qy ring of 6 never deadlocks.
            qy = {}

            def q_wgroup(wg):
                for w in wg:
                    qy[w] = [qypool.tile([hsz + 1 if hc == 4 else hsz, WIN],
                                         bf16, tag=f"qy_{hc}", name=f"qy_{hc}")
                             for hc, hsz in enumerate(HC_SIZES)]
                for hc, hsz in enumerate(HC_SIZES):
                    pqs = proj_round(Wq_sb, hc, hsz, wg, "pm")
                    for i, w in enumerate(wg):
                        nc.scalar.activation(
                            qy[w][hc][0:hsz, :], pqs[i][:],
                            AF.Sigmoid, bias=bq_sb[hc][:])
                        if hc == 4:
                            nc.vector.memset(qy[w][hc][hsz:hsz + 1, :], 1.0)

            # ---------- nd: num/den per batch chunk ----------
            # One 3-batch-wide op per (hc, chunk): es = E0*s (vector, Sx
            # broadcast along the batch axis), den = reduce(es), esv = es*v
            # (gpsimd), num = reduce(esv); then rr = num/den + bv.
            # V streams back from DRAM on gpsimd (sync stays clear for the
            # out writes).
            def nd_chunk(bs):
                nb, b0 = len(bs), bs[0]
                lo, hi = b0, b0 + nb
                vts = []
                for hc, hsz in enumerate(HC_SIZES):
                    vt = vrdpool.tile([hsz, nb * T], bf16, tag="vrd",
                                      name="vrd")
                    nc.gpsimd.dma_start(
                        vt[:], v_dram[hc * 128:hc * 128 + hsz,
                                      b0 * T:(b0 + nb) * T])
                    vts.append(vt)
                for hc, hsz in enumerate(HC_SIZES):
                    sA = scrpool.tile([hsz, nb * T], bf16, tag="sA", name="sA")
                    sB = scrpool.tile([hsz, nb * T], bf16, tag="sB", name="sB")
                    e3 = E0[hc][:, b0 * T:(b0 + nb) * T].rearrange(
                        "p (b t) -> p b t", t=T)
                    sA3 = sA[:].rearrange("p (b t) -> p b t", t=T)
                    sB3 = sB[:].rearrange("p (b t) -> p b t", t=T)
                    sx3 = Sx[hc][:, None, :].broadcast_to([hsz, nb, T])
                    nc.vector.tensor_mul(sA3, e3, sx3)
                    nc.vector.reduce_sum(den[hc][:, lo:hi], sA3,
                                         axis=mybir.AxisListType.X)
                    nc.gpsimd.tensor_mul(
                        sB3, sA3,
                        vts[hc][:].rearrange("p (b t) -> p b t", t=T))
                    nc.vector.reduce_sum(num[hc][:, lo:hi], sB3,
                                         axis=mybir.AxisListType.X)
                    nc.vector.reciprocal(rr[hc][:, lo:hi], den[hc][:, lo:hi])
                    nc.vector.tensor_mul(rr[hc][:, lo:hi], rr[hc][:, lo:hi],
                                         num[hc][:, lo:hi])
                    nc.vector.tensor_scalar_add(rr[hc][:, lo:hi],
                                                rr[hc][:, lo:hi],
                                                bv_sb[hc][:])

            # ---------- y = sigmoid(q) * r ; out = y_ext @ Wo_ext ----------
            def y_out(w):
                for b, lo, hi in _window_segments(w):
                    for hc, hsz in enumerate(HC_SIZES):
                        nc.vector.tensor_scalar_mul(
                            qy[w][hc][0:hsz, lo:hi],
                            qy[w][hc][0:hsz, lo:hi],
                            rr[hc][:, b:b + 1])
                for rc in range(NRC):
                    ob = obpool.tile([128, C], f32, tag="ob", name="ob")
                    poa = popool.tile([128, NOUT_HALF], f32, tag="po", name="po")
                    pob = popool.tile([128, NOUT_HALF], f32, tag="po", name="po")
                    for kc, ksz in enumerate(HC_SIZES):
                        psz = ksz + 1 if kc == 4 else ksz
                        lhs = qy[w][kc][0:psz, rc * 128:(rc + 1) * 128]
                        nc.tensor.matmul(
                            poa[:], lhs, Wo_sb[kc][0:psz, 0:NOUT_HALF],
                            start=(kc == 0), stop=(kc == 4))
                        nc.tensor.matmul(
                            pob[:], lhs, Wo_sb[kc][0:psz, NOUT_HALF:C],
                            start=(kc == 0), stop=(kc == 4))
                    nc.scalar.copy(ob[:, 0:NOUT_HALF], poa[:])
                    nc.vector.tensor_copy(ob[:, NOUT_HALF:C], pob[:])
                    nc.sync.dma_start(
                        out[w * WIN + rc * 128: w * WIN + (rc + 1) * 128, :],
                        ob[:])

            v_wgroup(WGROUPS[0])
            v_wgroup(WGROUPS[1])
            nd_chunk(BCHUNKS[0])
            v_wgroup(WGROUPS[2])
            nd_chunk(BCHUNKS[1])
            q_wgroup(WGROUPS[0])
            q_wgroup(WGROUPS[1])
            y_out(0)
            y_out(1)
            y_out(2)
            q_wgroup(WGROUPS[2])
            nd_chunk(BCHUNKS[2])
            y_out(3)
            y_out(4)
            y_out(5)
            y_out(6)
            y_out(7)
            y_out(8)

    nc.compile()
    return nc


def kernel(**inputs):
    global LAST_EXEC_NS
    from concourse import bass_utils
    import ml_dtypes

    if "nc" not in _CACHE:
        _CACHE["nc"] = _build()
    nc = _CACHE["nc"]

    x = np.asarray(inputs["x"], dtype=np.float32).reshape(B, T, C)

    def prep_wb(w):
        w = np.asarray(w, np.float32)
        w = w.reshape(NCC, 128, HID).transpose(1, 0, 2).reshape(128, NCC * HID)
        return np.ascontiguousarray(w).astype(ml_dtypes.bfloat16)

    eye = np.eye(128, dtype=ml_dtypes.bfloat16)
    common = {
        "Wqb": prep_wb(inputs["Wq"]),
        "Wkb": prep_wb(inputs["Wk"]),
        "Wvb": prep_wb(inputs["Wv"]),
        "bq": np.asarray(inputs["bq"], np.float32),
        "bk": np.asarray(inputs["bk"], np.float32),
        "bv": np.asarray(inputs["bv"], np.float32),
        "Wo": np.asarray(inputs["Wo"], np.float32),
        "bo": np.asarray(inputs["bo"], np.float32),
        "ident": eye,
    }
    in_maps = []
    for i in range(N_CORES):
        m = dict(common)
        m["x"] = np.ascontiguousarray(
            x[i * B_LOC:(i + 1) * B_LOC].reshape(R, C))
        in_maps.append(m)

    trace = bool(os.environ.get("KERNEL_TRACE"))
    res = bass_utils.run_bass_kernel_spmd(
        nc, in_maps, core_ids=list(range(N_CORES)), trace=trace)
    LAST_EXEC_NS = res.exec_time_ns

    shards = [res.results[i]["out"].reshape(B_LOC, Hh, Ww, C)
              for i in range(N_CORES)]
    return np.concatenate(shards, axis=0)
